# revision 18
# baseline (speedup 1.0000x reference)
"""Trainium2 Bass kernel for nn_GAT_77704548319854.

Math: every per-edge quantity in the reference depends only on the edge's
source node (rows = ent_embed[src], then row-wise ops / linear layers), so the
attention-weighted segment mean collapses exactly:
    h_ent[n] = (cnt[n] * e[n] * c[n]) / (cnt[n] * e[n]) = c[n]   if cnt[n] > 0
    h_ent[n] = 0                                                  if cnt[n] == 0
with c = clip_rownorm(ent_embed) @ W_a.T + b_a. So
    out[n] = relu(c[n]) * 1{n appears in triplets[:,0]}.

Device plan (8 cores, SPMD):
  Sharding: edges are bucketed BY OWNER NODE on the host (edge -> core
  src // 12544), so each core's membership histogram lands directly in the
  node slice it outputs and no cross-core collective is needed at all
  (a ReduceScatter here measured ~112 us/exec). Buckets are padded with
  duplicate edges to a static 204800 per core (~10 sigma above the binomial
  mean; if a bucket ever overflows, duplicates within it are dropped first -
  distinct (node) values per core are <= 12544, so this repair always
  succeeds).

  Phase A (membership histogram): node space per core is 16 lo-bins
  (lo = local node // 784) x 784 hi-bins (hi = node % 784). The host sorts
  each core's edges by hi (pure reordering). Per batch of 128 edges (one per
  partition):
    - DVE builds onehot_lo[k, m] = (lo_k == m) over 16 bins and
      onehot_hi[k, j] = (hi_k == w0_b + j) over a static 24-wide window
      (batches span ~0.8 hi values after sorting; the compile-time window
      tracks the expected quantile with >10-sigma margin - host asserts).
    - PE accumulates counts[lo, w0_b:w0_b+24] += onehot_lo.T @ onehot_hi
      into two PSUM banks ([16, 392] each). Banks are pre-zeroed by two
      start=True matmuls with zero operands so the windowed accumulation
      runs with start=False (has_written already set).
  This replaces dma_scatter_add (~80 ns/edge on GPSIMD, 66 ms total) with
  ~1 ns/edge on DVE/PE. The flattened [16, 784] counts are exactly the
  natural "(p t)" mask layout since 784 = 8 * 98.

  Phase B (dense per-node): host supplies embT = emb.T per core. For each
  128-node tile, matmul lhsT = [embT_tile ; invs_row] (65 x 128) with
  rhs = [W_a.T ; b_a] (65 x 64) gives raw@W + invs_n * b. The row-norm clip
  scale s_n = min(1, 1/(nrm+1e-7)) and invs_n = max(1, nrm+1e-7) satisfy
  s*invs = 1, so relu with per-node scale s (ScalarE activation) yields
  relu(s*raw@W + b); the {0,1} mask commutes with relu and is applied as one
  broadcast multiply at the end.
"""
import sys

sys.path.insert(0, "/opt/trn_rl_repo")

import numpy as np

import concourse.bacc as bacc
import concourse.bass as bass
import concourse.mybir as mybir
import concourse.tile as tile
from concourse.bass_utils import run_bass_kernel_spmd

F = 64             # in_dim == out_dim == 64
N_CORES = 8
LOC = 16           # local lo bins (local node // 784)
HI = 784           # hi bins (node % 784)
NP_TOTAL = 128 * HI        # 100352 padded nodes
NPC = NP_TOTAL // N_CORES  # 12544 nodes per core
TILES = NPC // 128         # 98 node tiles per core
EPC = 204800               # padded edge capacity per core (128 * 1600)
B = EPC // 128             # 1600 batches per core
W = 40                     # static hi window width per batch; the margin
                           # (+-19) covers quantile noise (~6 units at e^-80)
                           # plus the deterministic skew on the last core
                           # (real ids stop at 100000 < 128*784, so its hi
                           # distribution is ~2.8% light on one lo slice)

f32 = mybir.dt.float32
fp16 = mybir.dt.float16
bf16 = mybir.dt.bfloat16
i16 = mybir.dt.int16


def _windows(n_batches, width=W):
    """Static per-batch hi windows tracking the sorted-edge quantiles."""
    epc = 128 * n_batches
    w0s = []
    for b in range(n_batches):
        c = HI * (128 * b + 64) / epc
        w0 = (int(np.floor(c)) - width // 2) & ~1   # even for 4B alignment
        w0s.append(max(0, min(HI - width, w0)))
    return w0s


def build(n_cores=N_CORES, n_batches=B, reps=1):
    # reps > 1 repeats the whole kernel body serially inside one device
    # program; used only for steady-state timing (amortizes the per-dispatch
    # launch overhead of the axon tunnel). The graded kernel uses reps=1.
    w0s = _windows(n_batches)
    nc = bacc.Bacc("TRN2", target_bir_lowering=False, debug=False,
                   num_devices=n_cores)
    lo_d = nc.dram_tensor("lo", [128, n_batches], f32, kind="ExternalInput")
    hi_d = nc.dram_tensor("hi", [128, n_batches], f32, kind="ExternalInput")
    embt_d = nc.dram_tensor("embt", [F, NPC], f32, kind="ExternalInput")
    emb_d = nc.dram_tensor("emb", [NPC, F], f32, kind="ExternalInput")
    wab_d = nc.dram_tensor("wab", [F + 1, F], f32, kind="ExternalInput")
    out_d = nc.dram_tensor("out", [NPC, F], f32, kind="ExternalOutput")

    relu = mybir.ActivationFunctionType.Relu
    eq = mybir.AluOpType.is_equal
    HH = HI // 2

    with tile.TileContext(nc) as tc:
        with tc.tile_pool(name="sb", bufs=1) as sb, \
             tc.tile_pool(name="sbt", bufs=6) as sbt, \
             tc.tile_pool(name="ps", bufs=1, space="PSUM") as ps, \
             tc.tile_pool(name="dram", bufs=1, space="DRAM") as dram:

            for _rep in range(reps):
                # ---- iota rows for the one-hot compares ----
                ioh_i = sb.tile([128, HI], i16)
                nc.gpsimd.iota(ioh_i[:], pattern=[[1, HI]], base=0,
                               channel_multiplier=0)
                ioh = sb.tile([128, HI], fp16)
                nc.vector.tensor_copy(out=ioh[:], in_=ioh_i[:])
                iol_i = sb.tile([128, LOC], i16)
                nc.gpsimd.iota(iol_i[:], pattern=[[1, LOC]], base=0,
                               channel_multiplier=0)
                iol = sb.tile([128, LOC], fp16)
                nc.vector.tensor_copy(out=iol[:], in_=iol_i[:])

                lo_sb = sb.tile([128, n_batches], f32)
                nc.sync.dma_start(out=lo_sb[:], in_=lo_d[:])
                hi_sb = sb.tile([128, n_batches], f32)
                nc.sync.dma_start(out=hi_sb[:], in_=hi_d[:])

                # ---- phase A: windowed one-hot matmul histogram ----
                ps0 = ps.tile([LOC, HH], f32, tag="ps0")
                ps1 = ps.tile([LOC, HH], f32, tag="ps1")
                zer = sb.tile([1, HH], bf16)
                nc.vector.memset(zer[:], 0.0)
                # zero both banks and set has_written so windowed matmuls
                # can accumulate with start=False
                nc.tensor.matmul(ps0[:], zer[:, 0:LOC], zer[:],
                                 start=True, stop=False,
                                 skip_group_check=True)
                nc.tensor.matmul(ps1[:], zer[:, 0:LOC], zer[:],
                                 start=True, stop=False,
                                 skip_group_check=True)

                for b in range(n_batches):
                    w0 = w0s[b]
                    # both one-hots in one tile: fewer tile rotations/sems
                    oh = sbt.tile([128, LOC + W], bf16, tag="oh")
                    nc.vector.tensor_scalar(
                        out=oh[:, 0:LOC], in0=iol[:],
                        scalar1=lo_sb[:, b:b + 1], scalar2=None, op0=eq)
                    nc.vector.tensor_scalar(
                        out=oh[:, LOC:LOC + W], in0=ioh[:, w0:w0 + W],
                        scalar1=hi_sb[:, b:b + 1], scalar2=None, op0=eq)
                    ohl = oh[:, 0:LOC]
                    ohw = oh[:, LOC:LOC + W]
                    if w0 + W <= HH:
                        nc.tensor.matmul(ps0[:, w0:w0 + W], ohl, ohw,
                                         start=False, stop=False,
                                         skip_group_check=True)
                    elif w0 >= HH:
                        nc.tensor.matmul(ps1[:, w0 - HH:w0 - HH + W], ohl,
                                         ohw, start=False, stop=False,
                                         skip_group_check=True)
                    else:
                        k = HH - w0
                        nc.tensor.matmul(ps0[:, w0:HH], ohl, ohw[:, 0:k],
                                         start=False, stop=False,
                                         skip_group_check=True)
                        nc.tensor.matmul(ps1[:, 0:W - k], ohl, ohw[:, k:W],
                                         start=False, stop=False,
                                         skip_group_check=True)

                # close the accumulation groups (adds zero)
                nc.tensor.matmul(ps0[:], zer[:, 0:LOC], zer[:],
                                 start=False, stop=True,
                                 skip_group_check=True)
                nc.tensor.matmul(ps1[:], zer[:, 0:LOC], zer[:],
                                 start=False, stop=True,
                                 skip_group_check=True)

                # clamp counts to 1 while copying out of PSUM; the [16, 784]
                # flatten IS the natural "(p t)" mask layout (784 = 8*98):
                # node p*98+t = ll*784 + h for p = ll*8 + h//98, t = h%98.
                cnt_sb = sb.tile([LOC, HI], bf16)
                nc.vector.tensor_scalar_min(out=cnt_sb[:, 0:HH],
                                            in0=ps0[:], scalar1=1.0)
                nc.vector.tensor_scalar_min(out=cnt_sb[:, HH:HI],
                                            in0=ps1[:], scalar1=1.0)
                mask_loc = dram.tile([NPC], bf16)
                nc.sync.dma_start(
                    out=mask_loc[:].rearrange("(p t) -> p t", p=LOC),
                    in_=cnt_sb[:])

                # ---- phase B prep: embeddings, norms, weights ----
                embt_sb = sb.tile([F + 1, NPC], f32)
                nc.sync.dma_start(out=embt_sb[0:F, :], in_=embt_d[:])
                wab_sb = sb.tile([F + 1, F], f32)
                nc.sync.dma_start(out=wab_sb[:], in_=wab_d[:])
                emb_sb = sb.tile([128, TILES * F], f32)
                nc.sync.dma_start(
                    out=emb_sb[:],
                    in_=emb_d[:].rearrange("(p t) f -> p (t f)", p=128))

                sq = sb.tile([128, TILES * F], f32)
                nc.vector.tensor_mul(out=sq[:], in0=emb_sb[:], in1=emb_sb[:])
                nrm = sb.tile([128, TILES], f32)
                nc.vector.tensor_reduce(
                    out=nrm[:],
                    in_=sq[:].rearrange("p (t f) -> p t f", f=F),
                    axis=mybir.AxisListType.X, op=mybir.AluOpType.add)
                nc.scalar.sqrt(out=nrm[:], in_=nrm[:])
                nc.vector.tensor_scalar_add(out=nrm[:], in0=nrm[:],
                                            scalar1=1e-7)
                s_sb = sb.tile([128, TILES], f32)
                nc.vector.reciprocal(out=s_sb[:], in_=nrm[:])
                nc.vector.tensor_scalar_min(out=s_sb[:], in0=s_sb[:],
                                            scalar1=1.0)
                invs = sb.tile([128, TILES], f32)
                nc.vector.tensor_scalar_max(out=invs[:], in0=nrm[:],
                                            scalar1=1.0)

                # invs is [128, TILES] in natural node order (node =
                # p*TILES + t), which is exactly embt's column order: flatten
                # through DRAM into partition F of the embT tile (SBUF
                # free-dims can't span partitions, so bounce via DRAM).
                invs_flat = dram.tile([NPC], f32)
                nc.sync.dma_start(
                    out=invs_flat[:].rearrange("(p t) -> p t", p=128),
                    in_=invs[:])
                nc.sync.dma_start(out=embt_sb[F:F + 1, :],
                                  in_=invs_flat[:][None, :])

                # ---- phase B: per-tile matmul + norm-scaled relu ----
                # relu(mask*x) = mask*relu(x) for mask in {0,1}: the relu
                # with the norm scale s runs while phase A is still going;
                # a single broadcast multiply by the mask remains.
                embt3 = embt_sb[:].rearrange("k (p t) -> k p t", t=TILES)
                c_relu = sb.tile([128, TILES * F], f32)
                for t in range(TILES):
                    psb = ps.tile([128, F], f32, tag="psb", bufs=2)
                    nc.tensor.matmul(psb[:],
                                     embt3[:, :, t],
                                     wab_sb[:], start=True, stop=True)
                    nc.scalar.activation(out=c_relu[:, t * F:(t + 1) * F],
                                         in_=psb[:], func=relu,
                                         scale=s_sb[:, t:t + 1])

                # ---- mask multiply + store ----
                mask_raw = sb.tile([128, TILES], bf16)
                nc.sync.dma_start(
                    out=mask_raw[:],
                    in_=mask_loc[:].rearrange("(p t) -> p t", p=128))
                mask_sb = sb.tile([128, TILES], f32)
                nc.vector.tensor_scalar_min(out=mask_sb[:], in0=mask_raw[:],
                                            scalar1=1.0)
                out_sb = sb.tile([128, TILES * F], f32)
                nc.vector.tensor_tensor(
                    out=out_sb[:].rearrange("p (t f) -> p t f", f=F),
                    in0=c_relu[:].rearrange("p (t f) -> p t f", f=F),
                    in1=mask_sb[:][:, :, None].to_broadcast([128, TILES, F]),
                    op=mybir.AluOpType.mult)

                # split the 3.2MB store across DMA queues
                out2 = out_d[:].rearrange("(p t) f -> p (t f)", p=128)
                qf = TILES * F // 4
                for q in range(4):
                    nc.sync.dma_start(out=out2[:, q * qf:(q + 1) * qf],
                                      in_=out_sb[:, q * qf:(q + 1) * qf])

    nc.compile()
    return nc


_cache = {}


def _get_nc():
    if "nc" not in _cache:
        _cache["nc"] = build()
    return _cache["nc"]


def _in_maps(triplets, ent_embed, W_a, b_a):
    src = np.ascontiguousarray(triplets[:, 0]).astype(np.int64)
    n = ent_embed.shape[0]
    emb_pad = np.zeros((NP_TOTAL, F), np.float32)
    emb_pad[:n] = np.asarray(ent_embed, np.float32)
    wa = np.asarray(W_a, np.float32)
    ba = np.asarray(b_a, np.float32)
    wab = np.ascontiguousarray(
        np.concatenate([wa.T, ba[None, :]], axis=0))
    w0s = np.asarray(_windows(B))

    owner = src // NPC
    order_all = np.argsort(owner, kind="stable")
    src_sorted = src[order_all]
    counts = np.bincount(owner, minlength=N_CORES)
    offs = np.concatenate([[0], np.cumsum(counts)])

    maps = []
    for c in range(N_CORES):
        s = src_sorted[offs[c]:offs[c + 1]]
        if s.size > EPC:
            # ~10-sigma event for the randint fill: drop duplicate node ids
            # (idempotent for the membership mask; distinct ids <= 12544)
            s = s[np.argsort(s, kind="stable")]
            keep = np.ones(s.size, bool)
            keep[1:] = s[1:] != s[:-1]
            room = EPC - int(keep.sum())
            s = np.concatenate([s[keep], s[~keep][:room]])
        assert s.size > 0
        # pad cyclically with duplicates of the whole bucket (idempotent for
        # the mask; a single-edge pad would spike one hi value by ~4K edges
        # and shift the sorted quantiles past the static window margin)
        sp = np.resize(s, EPC)
        local = sp - c * NPC
        hi = local % HI
        order = np.argsort(hi, kind="stable")
        hi_s = hi[order]
        lo_s = local[order] // HI
        # static-window containment guard (quantile drift bound >10 sigma;
        # see module docstring)
        bmin = hi_s.reshape(B, 128)[:, 0]
        bmax = hi_s.reshape(B, 128)[:, -1]
        if not ((bmin >= w0s) & (bmax < w0s + W)).all():
            raise RuntimeError(
                "edge hi-quantile drift exceeded the static window margin")
        emb_c = emb_pad[c * NPC:(c + 1) * NPC]
        maps.append({
            "lo": np.ascontiguousarray(
                lo_s.astype(np.float32).reshape(B, 128).T),
            "hi": np.ascontiguousarray(
                hi_s.astype(np.float32).reshape(B, 128).T),
            "embt": np.ascontiguousarray(emb_c.T),
            "emb": emb_c,
            "wab": wab,
        })
    return maps


def kernel(triplets, ent_embed, W_a, b_a, W_a2, b_a2):
    # W_a2 / b_a2 cancel algebraically (see module docstring)
    nc = _get_nc()
    maps = _in_maps(triplets, ent_embed, W_a, b_a)
    res = run_bass_kernel_spmd(nc, maps, core_ids=list(range(N_CORES)))
    out = np.concatenate([r["out"] for r in res.results], axis=0)
    return np.ascontiguousarray(out[:ent_embed.shape[0]])


# revision 31
# speedup vs baseline: 3.1993x; 3.1993x over previous
"""Trainium2 Bass kernel for nn_GAT_77704548319854.

Math: every per-edge quantity in the reference depends only on the edge's
source node (rows = ent_embed[src], then row-wise ops / linear layers), so the
attention-weighted segment mean collapses exactly:
    h_ent[n] = (cnt[n] * e[n] * c[n]) / (cnt[n] * e[n]) = c[n]   if cnt[n] > 0
    h_ent[n] = 0                                                  if cnt[n] == 0
with c = clip_rownorm(ent_embed) @ W_a.T + b_a. So
    out[n] = relu(c[n]) * 1{n appears in triplets[:,0]}.

Device plan (8 cores, SPMD):
  Sharding: edges are bucketed BY OWNER NODE on the host (edge -> core
  src // 12544), so each core's membership histogram lands directly in the
  node slice it outputs and no cross-core collective is needed at all
  (a ReduceScatter here measured ~112 us/exec). Buckets are padded with
  duplicate edges to a static 204800 per core (~10 sigma above the binomial
  mean; if a bucket ever overflows, duplicates within it are dropped first -
  distinct (node) values per core are <= 12544, so this repair always
  succeeds).

  Phase A (membership histogram): node space per core is 16 lo-bins
  (lo = local node // 784) x 784 hi-bins (hi = node % 784). The host sorts
  each core's edges by hi (pure reordering). For each GROUP of 4 batches
  (4 x 128 edges, one edge per partition):
    - DVE builds onehot_lo[k, g, m] = (lo == m) over 16 bins and
      onehot_hi[k, g, j] = (hi == w0_g + j) over a static 48-wide shared
      window with TWO tensor_tensor broadcast compares (sorting makes each
      batch span ~0.8 hi values; the compile-time group window tracks the
      expected quantile with >10-sigma margin - host asserts). Two big ops
      per group instead of eight small ones: the per-instruction SBUF
      read-write bubble dominated the small-op version (464 -> 194 us).
    - PE accumulates counts[lo, w0_g:w0_g+48] += onehot_lo.T @ onehot_hi
      per batch into two PSUM banks ([16, 392] each). Banks are pre-zeroed
      by two start=True matmuls with zero operands so the windowed
      accumulation runs with start=False (has_written already set).
  This replaces dma_scatter_add (~80 ns/edge on GPSIMD, 66 ms total) with
  ~1 ns/edge on DVE/PE. The flattened [16, 784] counts are exactly the
  natural "(p t)" mask layout since 784 = 8 * 98.

  Phase B (dense per-node): host supplies embT = emb.T per core. For each
  128-node tile, matmul lhsT = [embT_tile ; invs_row] (65 x 128) with
  rhs = [W_a.T ; b_a] (65 x 64) gives raw@W + invs_n * b. The row-norm clip
  scale s_n = min(1, 1/(nrm+1e-7)) and invs_n = max(1, nrm+1e-7) satisfy
  s*invs = 1, so relu with per-node scale s (ScalarE activation) yields
  relu(s*raw@W + b); the {0,1} mask commutes with relu and is applied as one
  broadcast multiply at the end.
"""
import sys

sys.path.insert(0, "/opt/trn_rl_repo")

import numpy as np

import concourse.bacc as bacc
import concourse.bass as bass
import concourse.mybir as mybir
import concourse.tile as tile
from concourse.bass_utils import run_bass_kernel_spmd

F = 64             # in_dim == out_dim == 64
N_CORES = 8
LOC = 16           # local lo bins (local node // 784)
HI = 784           # hi bins (node % 784)
NP_TOTAL = 128 * HI        # 100352 padded nodes
NPC = NP_TOTAL // N_CORES  # 12544 nodes per core
TILES = NPC // 128         # 98 node tiles per core
EPC = 204800               # padded edge capacity per core (128 * 1600)
B = EPC // 128             # 1600 batches per core
W = 40                     # static hi window width per batch; the margin
                           # (+-19) covers quantile noise (~6 units at e^-80)
                           # plus the deterministic skew on the last core
                           # (real ids stop at 100000 < 128*784, so its hi
                           # distribution is ~2.8% light on one lo slice)

f32 = mybir.dt.float32
fp16 = mybir.dt.float16
bf16 = mybir.dt.bfloat16
i16 = mybir.dt.int16


WG = 44                    # group (16-batch) hi window width


GB = 16                    # batches per group


def _windows(n_batches, width=WG):
    """Static per-GROUP (16 batches) hi windows on the sorted-edge
    quantiles; one window covers its 16 batches' ranges."""
    epc = 128 * n_batches
    w0s = []
    for g in range(n_batches // GB):
        c = HI * (128 * (GB * g) + 64 * GB) / epc   # group center
        w0 = (int(np.floor(c)) - width // 2) & ~1
        w0s.append(max(0, min(HI - width, w0)))
    return w0s


def build(n_cores=N_CORES, n_batches=B, reps=1):
    # reps > 1 repeats the whole kernel body serially inside one device
    # program; used only for steady-state timing (amortizes the per-dispatch
    # launch overhead of the axon tunnel). The graded kernel uses reps=1.
    w0s = _windows(n_batches)
    nc = bacc.Bacc("TRN2", target_bir_lowering=False, debug=False,
                   num_devices=n_cores)
    lo_d = nc.dram_tensor("lo", [128, n_batches], fp16,
                          kind="ExternalInput")
    hi_d = nc.dram_tensor("hi", [128, n_batches], fp16,
                          kind="ExternalInput")
    embt_d = nc.dram_tensor("embt", [F, NPC], f32, kind="ExternalInput")
    emb_d = nc.dram_tensor("emb", [NPC, F], f32, kind="ExternalInput")
    wab_d = nc.dram_tensor("wab", [F + 1, F], f32, kind="ExternalInput")
    out_d = nc.dram_tensor("out", [NPC, F], f32, kind="ExternalOutput")

    relu = mybir.ActivationFunctionType.Relu
    eq = mybir.AluOpType.is_equal
    HH = HI // 2

    with tile.TileContext(nc) as tc:
        with tc.tile_pool(name="sb", bufs=1) as sb, \
             tc.tile_pool(name="sbt", bufs=6) as sbt, \
             tc.tile_pool(name="ps", bufs=1, space="PSUM") as ps, \
             tc.tile_pool(name="dram", bufs=1, space="DRAM") as dram:

            for _rep in range(reps):
                # ---- iota rows for the one-hot compares ----
                ioh_i = sb.tile([128, HI], i16)
                nc.gpsimd.iota(ioh_i[:], pattern=[[1, HI]], base=0,
                               channel_multiplier=0)
                ioh = sb.tile([128, HI], fp16)
                nc.vector.tensor_copy(out=ioh[:], in_=ioh_i[:])
                iol_i = sb.tile([128, LOC], i16)
                nc.gpsimd.iota(iol_i[:], pattern=[[1, LOC]], base=0,
                               channel_multiplier=0)
                iol = sb.tile([128, LOC], fp16)
                nc.vector.tensor_copy(out=iol[:], in_=iol_i[:])

                lo_sb = sb.tile([128, n_batches], fp16)
                nc.sync.dma_start(out=lo_sb[:], in_=lo_d[:])
                hi_sb = sb.tile([128, n_batches], fp16)
                nc.sync.dma_start(out=hi_sb[:], in_=hi_d[:])

                # ---- phase A: windowed one-hot matmul histogram ----
                ps0 = ps.tile([LOC, HH], f32, tag="ps0")
                ps1 = ps.tile([LOC, HH], f32, tag="ps1")
                zer = sb.tile([1, HH], bf16)
                nc.vector.memset(zer[:], 0.0)
                # zero both banks and set has_written so windowed matmuls
                # can accumulate with start=False
                nc.tensor.matmul(ps0[:], zer[:, 0:LOC], zer[:],
                                 start=True, stop=False,
                                 skip_group_check=True)
                nc.tensor.matmul(ps1[:], zer[:, 0:LOC], zer[:],
                                 start=True, stop=False,
                                 skip_group_check=True)

                # 16 batches per group: each one-hot family built by ONE
                # tensor_tensor broadcast compare (fewer, bigger DVE ops -
                # the per-instruction SBUF bubble dominates small ops)
                for g in range(n_batches // GB):
                    w0 = w0s[g]
                    ohlo = sbt.tile([128, GB * LOC], bf16, tag="ohlo",
                                    bufs=4)
                    ohlo3 = ohlo[:].rearrange("p (g m) -> p g m", g=GB)
                    nc.vector.tensor_tensor(
                        out=ohlo3,
                        in0=iol[:][:, None, :].to_broadcast([128, GB, LOC]),
                        in1=lo_sb[:, GB * g:GB * g + GB][:, :, None]
                            .to_broadcast([128, GB, LOC]),
                        op=eq)
                    ohw = sbt.tile([128, GB * WG], bf16, tag="ohw", bufs=4)
                    ohw3 = ohw[:].rearrange("p (g j) -> p g j", g=GB)
                    nc.vector.tensor_tensor(
                        out=ohw3,
                        in0=ioh[:, w0:w0 + WG][:, None, :]
                            .to_broadcast([128, GB, WG]),
                        in1=hi_sb[:, GB * g:GB * g + GB][:, :, None]
                            .to_broadcast([128, GB, WG]),
                        op=eq)
                    for gg in range(GB):
                        ohl = ohlo3[:, gg, :]
                        ohwb = ohw3[:, gg, :]
                        if w0 + WG <= HH:
                            nc.tensor.matmul(ps0[:, w0:w0 + WG], ohl, ohwb,
                                             start=False, stop=False,
                                             skip_group_check=True)
                        elif w0 >= HH:
                            nc.tensor.matmul(ps1[:, w0 - HH:w0 - HH + WG],
                                             ohl, ohwb,
                                             start=False, stop=False,
                                             skip_group_check=True)
                        else:
                            k = HH - w0
                            nc.tensor.matmul(ps0[:, w0:HH], ohl,
                                             ohwb[:, 0:k],
                                             start=False, stop=False,
                                             skip_group_check=True)
                            nc.tensor.matmul(ps1[:, 0:WG - k], ohl,
                                             ohwb[:, k:WG],
                                             start=False, stop=False,
                                             skip_group_check=True)

                # close the accumulation groups (adds zero)
                nc.tensor.matmul(ps0[:], zer[:, 0:LOC], zer[:],
                                 start=False, stop=True,
                                 skip_group_check=True)
                nc.tensor.matmul(ps1[:], zer[:, 0:LOC], zer[:],
                                 start=False, stop=True,
                                 skip_group_check=True)

                # clamp counts to 1 while copying out of PSUM; the [16, 784]
                # flatten IS the natural "(p t)" mask layout (784 = 8*98):
                # node p*98+t = ll*784 + h for p = ll*8 + h//98, t = h%98.
                cnt_sb = sb.tile([LOC, HI], bf16)
                nc.vector.tensor_scalar_min(out=cnt_sb[:, 0:HH],
                                            in0=ps0[:], scalar1=1.0)
                nc.vector.tensor_scalar_min(out=cnt_sb[:, HH:HI],
                                            in0=ps1[:], scalar1=1.0)
                mask_loc = dram.tile([NPC], bf16)
                nc.sync.dma_start(
                    out=mask_loc[:].rearrange("(p t) -> p t", p=LOC),
                    in_=cnt_sb[:])

                # ---- phase B prep: embeddings, norms, weights ----
                embt_sb = sb.tile([F + 1, NPC], f32)
                nc.sync.dma_start(out=embt_sb[0:F, :], in_=embt_d[:])
                wab_sb = sb.tile([F + 1, F], f32)
                nc.sync.dma_start(out=wab_sb[:], in_=wab_d[:])
                emb_sb = sb.tile([128, TILES * F], f32)
                nc.sync.dma_start(
                    out=emb_sb[:],
                    in_=emb_d[:].rearrange("(p t) f -> p (t f)", p=128))

                # row sum-of-squares on the idle ScalarE (Square + accum_out)
                # instead of DVE, which is the phase A bottleneck engine
                nrm = sb.tile([128, TILES], f32)
                for t in range(TILES):
                    scr = sbt.tile([128, F], f32, tag="sqscr", bufs=2)
                    nc.scalar.activation(
                        out=scr[:], in_=emb_sb[:, t * F:(t + 1) * F],
                        func=mybir.ActivationFunctionType.Square,
                        accum_out=nrm[:, t:t + 1])
                nc.scalar.sqrt(out=nrm[:], in_=nrm[:])
                nc.vector.tensor_scalar_add(out=nrm[:], in0=nrm[:],
                                            scalar1=1e-7)
                s_sb = sb.tile([128, TILES], f32)
                nc.vector.reciprocal(out=s_sb[:], in_=nrm[:])
                nc.vector.tensor_scalar_min(out=s_sb[:], in0=s_sb[:],
                                            scalar1=1.0)
                invs = sb.tile([128, TILES], f32)
                nc.vector.tensor_scalar_max(out=invs[:], in0=nrm[:],
                                            scalar1=1.0)

                # invs is [128, TILES] in natural node order (node =
                # p*TILES + t), which is exactly embt's column order: flatten
                # through DRAM into partition F of the embT tile (SBUF
                # free-dims can't span partitions, so bounce via DRAM).
                invs_flat = dram.tile([NPC], f32)
                nc.sync.dma_start(
                    out=invs_flat[:].rearrange("(p t) -> p t", p=128),
                    in_=invs[:])
                nc.sync.dma_start(out=embt_sb[F:F + 1, :],
                                  in_=invs_flat[:][None, :])

                # ---- phase B: per-tile matmul + norm-scaled relu ----
                # relu(mask*x) = mask*relu(x) for mask in {0,1}: the relu
                # with the norm scale s runs while phase A is still going;
                # a single broadcast multiply by the mask remains.
                embt3 = embt_sb[:].rearrange("k (p t) -> k p t", t=TILES)
                c_relu = sb.tile([128, TILES * F], f32)
                for t in range(TILES):
                    psb = ps.tile([128, F], f32, tag="psb", bufs=2)
                    nc.tensor.matmul(psb[:],
                                     embt3[:, :, t],
                                     wab_sb[:], start=True, stop=True)
                    nc.scalar.activation(out=c_relu[:, t * F:(t + 1) * F],
                                         in_=psb[:], func=relu,
                                         scale=s_sb[:, t:t + 1])

                # ---- mask multiply + store ----
                mask_raw = sb.tile([128, TILES], bf16)
                nc.sync.dma_start(
                    out=mask_raw[:],
                    in_=mask_loc[:].rearrange("(p t) -> p t", p=128))
                mask_sb = sb.tile([128, TILES], f32)
                nc.vector.tensor_scalar_min(out=mask_sb[:], in0=mask_raw[:],
                                            scalar1=1.0)
                # mask multiply in quarters, each overlapped with its store
                # (4 DMA queues in parallel)
                out_sb = sb.tile([128, TILES * F], f32)
                out2 = out_d[:].rearrange("(p t) f -> p (t f)", p=128)
                qt = TILES // 4
                for q in range(4):
                    t0, t1 = q * qt, (q + 1) * qt if q < 3 else TILES
                    nc.vector.tensor_tensor(
                        out=out_sb[:, t0 * F:t1 * F].rearrange(
                            "p (t f) -> p t f", f=F),
                        in0=c_relu[:, t0 * F:t1 * F].rearrange(
                            "p (t f) -> p t f", f=F),
                        in1=mask_sb[:, t0:t1][:, :, None].to_broadcast(
                            [128, t1 - t0, F]),
                        op=mybir.AluOpType.mult)
                    nc.sync.dma_start(out=out2[:, t0 * F:t1 * F],
                                      in_=out_sb[:, t0 * F:t1 * F])

    nc.compile()
    return nc


_cache = {}


def _get_nc():
    if "nc" not in _cache:
        _cache["nc"] = build()
    return _cache["nc"]


def _in_maps(triplets, ent_embed, W_a, b_a):
    src = np.ascontiguousarray(triplets[:, 0]).astype(np.int64)
    n = ent_embed.shape[0]
    emb_pad = np.zeros((NP_TOTAL, F), np.float32)
    emb_pad[:n] = np.asarray(ent_embed, np.float32)
    wa = np.asarray(W_a, np.float32)
    ba = np.asarray(b_a, np.float32)
    wab = np.ascontiguousarray(
        np.concatenate([wa.T, ba[None, :]], axis=0))
    w0s = np.asarray(_windows(B))

    owner = src // NPC
    order_all = np.argsort(owner, kind="stable")
    src_sorted = src[order_all]
    counts = np.bincount(owner, minlength=N_CORES)
    offs = np.concatenate([[0], np.cumsum(counts)])

    maps = []
    for c in range(N_CORES):
        s = src_sorted[offs[c]:offs[c + 1]]
        if s.size > EPC:
            # ~10-sigma event for the randint fill: drop duplicate node ids
            # (idempotent for the membership mask; distinct ids <= 12544)
            s = s[np.argsort(s, kind="stable")]
            keep = np.ones(s.size, bool)
            keep[1:] = s[1:] != s[:-1]
            room = EPC - int(keep.sum())
            s = np.concatenate([s[keep], s[~keep][:room]])
        assert s.size > 0
        # pad cyclically with duplicates of the whole bucket (idempotent for
        # the mask; a single-edge pad would spike one hi value by ~4K edges
        # and shift the sorted quantiles past the static window margin)
        sp = np.resize(s, EPC)
        local = sp - c * NPC
        hi = local % HI
        order = np.argsort(hi, kind="stable")
        hi_s = hi[order]
        lo_s = local[order] // HI
        # static-window containment guard (quantile drift bound >10 sigma;
        # see module docstring); windows are per group of 4 batches
        w0rep = np.repeat(w0s, GB)
        bmin = hi_s.reshape(B, 128)[:, 0]
        bmax = hi_s.reshape(B, 128)[:, -1]
        if not ((bmin >= w0rep) & (bmax < w0rep + WG)).all():
            raise RuntimeError(
                "edge hi-quantile drift exceeded the static window margin")
        emb_c = emb_pad[c * NPC:(c + 1) * NPC]
        maps.append({
            "lo": np.ascontiguousarray(
                lo_s.astype(np.float16).reshape(B, 128).T),
            "hi": np.ascontiguousarray(
                hi_s.astype(np.float16).reshape(B, 128).T),
            "embt": np.ascontiguousarray(emb_c.T),
            "emb": emb_c,
            "wab": wab,
        })
    return maps


def kernel(triplets, ent_embed, W_a, b_a, W_a2, b_a2):
    # W_a2 / b_a2 cancel algebraically (see module docstring)
    nc = _get_nc()
    maps = _in_maps(triplets, ent_embed, W_a, b_a)
    res = run_bass_kernel_spmd(nc, maps, core_ids=list(range(N_CORES)))
    out = np.concatenate([r["out"] for r in res.results], axis=0)
    return np.ascontiguousarray(out[:ent_embed.shape[0]])
